# revision 6
# baseline (speedup 1.0000x reference)
"""Trainium2 Bass kernel for nn_MultiHeadAttention_68659347194437.

Spatial multi-head attention over the W axis (no softmax):
    qp = wq*q + bq ; kp, vp likewise            (1x1 conv over C=32)
    attn = qp @ kp^T  per (b,h)                 [512, 512]
    att  = attn @ vp                            [512, 32]
    out  = att^T + q                            (NCHW residual)

No softmax, so associativity collapses the [512,512] score matrix.
Per head:
    G   = V_aug^T K_aug                  [33,33]  (K_aug = [K; ones])
    A   = [wq|bq]^T [wk|bk]              [33,33]  (constant)
    L   = A (G^T Pv^T) + [I;0]           [33,32]  (Pv = [wv|bv]; +I = residual)
    out = L^T Q_aug                      [32,512]
The L^T Q_aug step dominates PE time if done per head (free dim 512 at
25% array util).  Instead, for each block of 4 heads, the top 32 rows of
L are scattered into a block-diagonal [128,128] stationary L4, and one
matmul computes all 4 heads: out4 = L4^T Q4  (Q4 = 4 heads' channels
stacked on partitions).  The bias row of L (driven by the ones-row of
Q_aug, which no longer fits) is applied as a per-partition scalar add
fused into the PSUM->SBUF output copy.

Sharding: data-parallel over batch B=8 across 8 NeuronCores, no comms.
All input groups get unique SBUF buffers (everything fits), so input
DMAs have no reuse waits and are issued upfront on three queues
(sync=K, gpsimd=V, vector=Q); outputs alternate between the gpsimd and
sync queues.  The per-block compute is software-pipelined 4 deep so the
PE never waits on a PSUM->SBUF copy round trip.
"""

import os
import numpy as np

import concourse.bass as bass
import concourse.bacc as bacc
import concourse.tile as tile
import concourse.mybir as mybir
from concourse.bass_utils import run_bass_kernel_spmd

B, C, H, W = 8, 32, 64, 512
CA = C + 1          # augmented channel dim (ones row/col)
HW = H * W
NCHUNK = HW // 128  # 256 chunks of 128 pixels (4 per head)
NBLK = H // 4       # 16 blocks of 4 heads
GROUPS = [4, 4, 8, 16, 16, 16]  # heads per input-DMA group

BF16 = mybir.dt.bfloat16
F32 = mybir.dt.float32
NP_BF16 = np.dtype(mybir.dt.np(BF16))

# exec time (ns) of the most recent run, when tracing was enabled
last_exec_time_ns = None

_cache = {}


def _build():
    nc = bacc.Bacc(
        "TRN2",
        target_bir_lowering=False,
        debug=False,
        enable_asserts=False,
        num_devices=8,
    )

    q4_d = nc.dram_tensor("q4", [128, NBLK * W], BF16, kind="ExternalInput")
    kt_d = nc.dram_tensor("kta", [128, NCHUNK * CA], BF16, kind="ExternalInput")
    vt_d = nc.dram_tensor("vta", [128, NCHUNK * CA], BF16, kind="ExternalInput")
    # cf: [wq|bq | wk|bk | I33] f32;  cb: [pvt | I32 | I32 tiled x4] bf16
    cf_d = nc.dram_tensor("cf", [CA, 3 * CA], F32, kind="ExternalInput")
    cb_d = nc.dram_tensor("cb", [CA, 2 * C + 128], BF16, kind="ExternalInput")
    out_d = nc.dram_tensor("out", [128, NBLK, W], BF16, kind="ExternalOutput")

    q4 = q4_d.ap()
    kta = kt_d.ap()
    vta = vt_d.ap()
    out_ap = out_d.ap().rearrange("p a w -> p (a w)")

    # block -> (group idx, block-in-group)
    blk2grp = []
    for g, ghn in enumerate(GROUPS):
        for blk in range(ghn // 4):
            blk2grp.append((g, blk))
    assert len(blk2grp) == NBLK

    with tile.TileContext(nc) as tc:
        with (
            tc.tile_pool(name="const", bufs=1) as cpool,
            tc.tile_pool(name="qin", bufs=1) as qpool,
            tc.tile_pool(name="kvin", bufs=1) as kvpool,
            tc.tile_pool(name="l4p", bufs=1) as l4pool,
            tc.tile_pool(name="outp", bufs=1) as opool,
            tc.tile_pool(name="small", bufs=4) as spool,
            tc.tile_pool(name="psg", bufs=2, space=bass.MemorySpace.PSUM) as psg,
            tc.tile_pool(name="psy", bufs=2, space=bass.MemorySpace.PSUM) as psy,
            tc.tile_pool(name="psl", bufs=2, space=bass.MemorySpace.PSUM) as psl,
            tc.tile_pool(name="pso", bufs=2, space=bass.MemorySpace.PSUM) as pso,
        ):
            # ---- constants: one f32 + one bf16 blob, first on the sync queue ----
            cf = cpool.tile([CA, 3 * CA], F32)
            cb = cpool.tile([CA, 2 * C + 128], BF16)
            nc.sync.dma_start(cf[:], cf_d.ap()[:])
            nc.sync.dma_start(cb[:], cb_d.ap()[:])
            wqb = cf[0:C, 0:CA]
            wkb = cf[0:C, CA:2 * CA]
            i33 = cf[:, 2 * CA:3 * CA]
            pvt = cb[:, 0:C]                  # [33, 32] = [wv^T; bv]
            ieye = cb[0:C, C:2 * C]           # [32, 32] identity
            i32t = cb[0:C, 2 * C:2 * C + 128]  # [32, 128] = [I I I I]

            # ---- input DMAs: all groups upfront, unique buffers ----
            # sync: K
            ktgs, vtgs, qgs = [], [], []
            h0 = 0
            for g, ghn in enumerate(GROUPS):
                ktg = kvpool.tile([128, ghn * 4 * CA], BF16,
                                  tag=f"ktg{g}", name=f"ktg{g}")
                nc.sync.dma_start(ktg[:], kta[:, h0 * 4 * CA:(h0 + ghn) * 4 * CA])
                ktgs.append(ktg)
                h0 += ghn
            # gpsimd: early L4 memsets (clears off-diagonals once), then V,
            # then the remaining memsets
            l4s = [
                l4pool.tile([128, 128], BF16, tag=f"l4_{b}", name=f"l4b{b}")
                for b in range(NBLK)
            ]
            for b in range(4):
                nc.gpsimd.memset(l4s[b][:], 0.0)
            h0 = 0
            for g, ghn in enumerate(GROUPS):
                vtg = kvpool.tile([128, ghn * 4 * CA], BF16,
                                  tag=f"vtg{g}", name=f"vtg{g}")
                nc.gpsimd.dma_start(vtg[:], vta[:, h0 * 4 * CA:(h0 + ghn) * 4 * CA])
                vtgs.append(vtg)
                h0 += ghn
            for b in range(4, NBLK):
                nc.gpsimd.memset(l4s[b][:], 0.0)
            # scalar: Q (g0-g2 upfront, g3-g5 interleaved into the loop)
            q4_offs = []
            h0 = 0
            for g, ghn in enumerate(GROUPS):
                qg = qpool.tile([128, (ghn // 4) * W], BF16,
                                tag=f"qg{g}", name=f"qg{g}")
                q4_offs.append(((h0 // 4) * W, (h0 // 4 + ghn // 4) * W))
                if g < 3:
                    nc.scalar.dma_start(qg[:], q4[:, q4_offs[g][0]:q4_offs[g][1]])
                qgs.append(qg)
                h0 += ghn

            # ---- A = [wq|bq]^T [wk|bk]; at_sb = A^T (PE transpose via I) ----
            a_ps = psg.tile([CA, CA], F32, tag="g")
            nc.tensor.matmul(a_ps[:], wqb, wkb)
            a_sb = cpool.tile([CA, CA], F32)
            nc.vector.tensor_copy(a_sb[:], a_ps[:])
            at_ps = psg.tile([CA, CA], F32, tag="g")
            nc.tensor.matmul(at_ps[:], a_sb[:], i33)
            at_sb = cpool.tile([CA, CA], BF16)
            nc.vector.tensor_copy(at_sb[:], at_ps[:])

            # ---- software-pipelined main loop ----
            # iteration t: G(t), m1(t-1), l~+bias(t-2), out(t-3)+copies+DMA
            gt_ps_t = {}
            gt_sb_t = {}
            m1_ps_t = {}
            m1b_t = {}
            lb_ps_t = {}
            bias_t = {}
            o_ps_t = {}

            for t in range(NBLK + 3):
                # --- stage A: G for block t ---
                if t < NBLK:
                    g, blk = blk2grp[t]
                    ktg, vtg = ktgs[g], vtgs[g]
                    gt_ps = psg.tile([CA, 4 * CA], F32, tag="g")
                    for i in range(4):
                        o0 = ((blk * 4 + i) * 4) * CA
                        for j in range(4):
                            o = o0 + j * CA
                            nc.tensor.matmul(
                                gt_ps[:, i * CA:(i + 1) * CA],
                                vtg[:, o:o + CA],
                                ktg[:, o:o + CA],
                                start=(j == 0),
                                stop=(j == 3),
                            )
                    gt_ps_t[t] = gt_ps

                # --- stage B: m1 = G^T Pv^T (4 heads) for block t-1 ---
                tb = t - 1
                if 0 <= tb < NBLK:
                    m1_ps = psy.tile([CA, 4 * C], F32, tag="m1")
                    gt_sb = gt_sb_t[tb]
                    for i in range(4):
                        nc.tensor.matmul(
                            m1_ps[:, i * C:(i + 1) * C],
                            gt_sb[:, i * CA:(i + 1) * CA],
                            pvt,
                        )
                    m1_ps_t[tb] = m1_ps

                # --- stage C: l~ = I + (A m1)[:32,:], bias = (A m1)[32,:] ---
                tl = t - 2
                if 0 <= tl < NBLK:
                    lb_ps = psl.tile([128, 4 * C + 4], F32, tag="l")
                    m1b = m1b_t[tl]
                    nc.tensor.matmul(lb_ps[0:C, 0:4 * C], ieye, i32t,
                                     start=True, stop=False)
                    nc.tensor.matmul(lb_ps[0:C, 0:4 * C], at_sb[:, 0:C], m1b[:],
                                     start=False, stop=True)
                    nc.tensor.matmul(lb_ps[:, 4 * C:4 * C + 1], m1b[:],
                                     at_sb[:, C:C + 1])
                    lb_ps_t[tl] = lb_ps

                # --- stage D: out4 = L4^T Q4 for block t-3 ---
                to = t - 3
                if 0 <= to < NBLK:
                    g, blk = blk2grp[to]
                    o_ps = pso.tile([128, W], F32, tag="o")
                    nc.tensor.matmul(o_ps[:], l4s[to][:],
                                     qgs[g][:, blk * W:(blk + 1) * W])
                    o_ps_t[to] = o_ps

                # --- copies (emitted after PE so same-queue order is right) ---
                if 0 <= tl < NBLK:
                    lb_ps = lb_ps_t[tl]
                    l4 = l4s[tl]
                    for i in range(4):
                        src = lb_ps[0:C, C * i:C * (i + 1)]
                        dst = l4[C * i:C * (i + 1), C * i:C * (i + 1)]
                        if i < 2:
                            nc.vector.tensor_copy(dst, src)
                        else:
                            nc.scalar.copy(dst, src)
                    bias_sb = spool.tile([128, 1], F32, tag="bias")
                    nc.vector.tensor_copy(bias_sb[:], lb_ps[:, 4 * C:4 * C + 1])
                    bias_t[tl] = bias_sb

                if t < NBLK:
                    gt_sb = spool.tile([CA, 4 * CA], BF16, tag="gt")
                    nc.scalar.copy(gt_sb[:], gt_ps_t[t][:])
                    gt_sb_t[t] = gt_sb
                if t < 3:
                    g = t + 3
                    nc.scalar.dma_start(qgs[g][:], q4[:, q4_offs[g][0]:q4_offs[g][1]])

                if 0 <= tb < NBLK:
                    m1b = spool.tile([CA, 4 * C], BF16, tag="m1b")
                    nc.vector.tensor_copy(m1b[:], m1_ps_t[tb][:])
                    m1b_t[tb] = m1b

                # --- output copy (+bias) and DMA for block t-3 ---
                if 0 <= to < NBLK:
                    og = opool.tile([128, W], BF16, tag=f"og{to}", name=f"og{to}")
                    o_ps = o_ps_t[to]
                    if to % 2 == 0:
                        nc.vector.tensor_scalar_add(og[:], o_ps[:], bias_t[to][:])
                        nc.gpsimd.dma_start(out_ap[:, to * W:(to + 1) * W], og[:])
                    else:
                        nc.scalar.add(og[:], o_ps[:], bias_t[to][:])
                        nc.sync.dma_start(out_ap[:, to * W:(to + 1) * W], og[:])

    nc.compile()
    return nc


def _prep_core(qb, kb, vb):
    """Host-side relayout for one batch element (one core)."""
    # Q: 4-head channel stacking  [128, NBLK*W], partition = 32*(h%4) + c
    q4 = np.ascontiguousarray(
        qb.reshape(C, NBLK, 4, W).transpose(2, 0, 1, 3)
    ).reshape(128, NBLK * W).astype(NP_BF16)

    def tr(x):
        t = np.empty((HW, CA), dtype=np.float32)
        t[:, :C] = x.reshape(C, HW).T
        t[:, C] = 1.0
        return np.ascontiguousarray(
            t.reshape(NCHUNK, 128, CA).transpose(1, 0, 2)
        ).reshape(128, NCHUNK * CA).astype(NP_BF16)

    return q4, tr(kb), tr(vb)


def _install_ntff_hook():
    """Provide antenv.axon_hooks (absent in this image) so trace=True works."""
    import sys
    import types

    if "antenv.axon_hooks" in sys.modules:
        return
    try:
        import antenv
    except ImportError:
        return
    mod = types.ModuleType("antenv.axon_hooks")
    store = {}
    mod.set_axon_ntff_profile_hook = lambda h: store.__setitem__("h", h)
    mod.get_axon_ntff_profile_hook = lambda: store.get("h")
    sys.modules["antenv.axon_hooks"] = mod
    antenv.axon_hooks = mod
    try:
        from trn_agent_boot.trn_boot import _ntff_profile_via_ctypes

        hook = _ntff_profile_via_ctypes("/opt/axon/libaxon_pjrt.so")
        if hook is not None:
            store["h"] = hook
    except Exception:
        pass


def kernel(q, k, v, wq, bq, wk, bk, wv, bv):
    global last_exec_time_ns
    if "nc" not in _cache:
        _cache["nc"] = _build()
    nc = _cache["nc"]

    q = np.asarray(q, np.float32)
    k = np.asarray(k, np.float32)
    v = np.asarray(v, np.float32)
    wq = np.asarray(wq, np.float32)
    bq = np.asarray(bq, np.float32)
    wk = np.asarray(wk, np.float32)
    bk = np.asarray(bk, np.float32)
    wv = np.asarray(wv, np.float32)
    bv = np.asarray(bv, np.float32)

    cf = np.zeros((CA, 3 * CA), np.float32)
    cf[0:C, 0:CA] = np.concatenate([wq, bq[:, None]], axis=1)
    cf[0:C, CA:2 * CA] = np.concatenate([wk, bk[:, None]], axis=1)
    cf[:, 2 * CA:3 * CA] = np.eye(CA)
    cb = np.zeros((CA, 2 * C + 128), np.float32)
    cb[:, 0:C] = np.concatenate([wv.T, bv[None, :]], axis=0)
    cb[0:C, C:2 * C] = np.eye(C)
    cb[0:C, 2 * C:2 * C + 128] = np.tile(np.eye(C), (1, 4))
    cb = cb.astype(NP_BF16)

    in_maps = []
    for b in range(B):
        q4, kta, vta = _prep_core(q[b], k[b], v[b])
        in_maps.append({
            "q4": q4, "kta": kta, "vta": vta,
            "cf": cf, "cb": cb,
        })

    trace = os.environ.get("KERNEL_TRACE", "0") == "1"
    if trace:
        _install_ntff_hook()
    res = run_bass_kernel_spmd(nc, in_maps, core_ids=list(range(B)), trace=trace)
    last_exec_time_ns = res.exec_time_ns

    outs = []
    for b in range(B):
        arr = np.asarray(res.results[b]["out"], dtype=np.float32).reshape(4, C, H // 4, W)
        outs.append(np.transpose(arr, (1, 2, 0, 3)).reshape(C, H, W))
    return np.stack(outs).astype(np.float32)
